# revision 10
# baseline (speedup 1.0000x reference)
"""KAN feed-forward on Trainium2 — Bass/Tile kernel, 8-core data-parallel, v2.

Math: each KAN layer is y = silu(x) @ scale_base + spline(x) with the spline
an 8-coef cubic B-spline per edge on u = 2.5x + 5.5 (knots at integers).
Using the truncated-power representation and splitting the B-spline stencil
Delta^4 = Delta^1 (weights) o Delta^3 (features):

    y_sp[n,o] = sum_{g=0..8} D_g(u_ni) * V[i,o,g]
    D_g(u)    = r_g - 3 r_{g+1} + 3 r_{g+2} - r_{g+3},  r_m = relu(u-m)^3
    V[:,:,g]  = (A[:,:,g] - A[:,:,g-1]) / 6,   A = coef*scale_sp, A_{-1}=A_8=0

This is an EXACT identity on all of R (Delta^3 of a cubic is the constant 6
and sum_g V_g = 0 telescopes), so u needs no clamping.  D_g is a smooth step
bounded in [0,6], so fp16 matmul operands are numerically safe (measured
end-to-end rel-err ~3e-3 vs the 2e-2 gate) and the PE runs at full 16-bit
rate.  The f32 truncated-power formulation (v1) streams the PE 4x slower and
cannot be quantized: operands up to 11^3 cancel to ~0.1.

Per-core layout (512 tokens/core):
  L1: out1[h, tok] over 40 K-tiles (4 silu + 36 D-planes), run as two bank
      phases (h 0..511 then 512..1023, weights streamed as column halves) so
      pb[0..3] complete at half-L1 and the L2 feature chains overlap phase B.
  L2: out2[tok, o] over 88 K-tiles (per h-block j: silu + 9 D-planes),
      j-group-major; L1's PSUM output [h, tok] feeds the chains directly.
Feature chains: s_m = Relu(2.5x+5.5-m) and q_m = Square(2.5x+5.5-m) on ACT,
r = q*s, then the difference chain d/f/D (Delta^1 three times) on DVE — ~23 ACT + 39 DVE ops per 128-dim input block, balanced engines.
"""

import os
import sys
from contextlib import ExitStack

import numpy as np

for _p in ("/opt/trn_rl_repo",):
    if _p not in sys.path:
        sys.path.insert(0, _p)

# ---------------------------------------------------------------- constants
NG = 8                      # B-spline coefficients per edge
NP = 9                      # D (Delta^3 step) planes per input dim
D, H, O = 512, 1024, 512
NCORES = 8
NTOK = 4096
TOK = NTOK // NCORES        # 512 tokens per core
P = 128

L1_NK = 4 + NP * 4          # 40 K-tiles of 128 (4 silu + 36 D)
L2_NK = 8 * (1 + NP)        # 88 K-tiles of 128 (8 silu + 72 D)

W_MODE = os.environ.get("KAN_W_DT", "f16")
R_MODE = os.environ.get("KAN_R_DT", "f16")

_BUILD_CACHE: dict = {}


def _np_wdt():
    if W_MODE == "bf16":
        import ml_dtypes

        return ml_dtypes.bfloat16
    if W_MODE == "f16":
        return np.float16
    return np.float32


# ---------------------------------------------------------------- host prep
def _vweights(coef, scale_sp) -> np.ndarray:
    """V[:, :, g] = (A_g - A_{g-1})/6 for g = 0..8, A = coef*scale_sp."""
    A = coef.astype(np.float64) * scale_sp.astype(np.float64)[:, :, None]
    z = np.zeros_like(A[:, :, :1])
    V = (np.concatenate([A, z], 2) - np.concatenate([z, A], 2)) / 6.0
    return V  # (Din, Dout, 9)


def _pack_w1(coef1, scale_sp1, scale_base1) -> np.ndarray:
    """-> (2, 40, 128, 512): [hidden-col half][K-tile][K rows][cols]."""
    V1 = _vweights(coef1, scale_sp1)  # (512, 1024, 9)
    w1 = np.empty((2, L1_NK, P, 512), np.float32)
    for half in range(2):
        cs = slice(half * 512, (half + 1) * 512)
        for ib in range(4):
            w1[half, ib] = scale_base1[ib * P : (ib + 1) * P, cs]
        for ib in range(4):
            for g in range(NP):
                w1[half, 4 + ib * NP + g] = V1[ib * P : (ib + 1) * P, cs, g]
    return np.ascontiguousarray(w1.astype(_np_wdt()))


def _pack_w2(coef2, scale_sp2, scale_base2) -> np.ndarray:
    """-> (88, 128, 512): per h-block j: [silu, D_0..D_8] K-tiles."""
    V2 = _vweights(coef2, scale_sp2)  # (1024, 512, 9)
    w2 = np.empty((L2_NK, P, O), np.float32)
    for j in range(8):
        rs = slice(j * P, (j + 1) * P)
        w2[j * (1 + NP)] = scale_base2[rs]
        for g in range(NP):
            w2[j * (1 + NP) + 1 + g] = V2[rs, :, g]
    return np.ascontiguousarray(w2.astype(_np_wdt()))


# ---------------------------------------------------------------- bass build
def _build_kernel():
    if "nc" in _BUILD_CACHE:
        return _BUILD_CACHE["nc"]

    import concourse.mybir as mybir
    import concourse.tile as tile
    from concourse import bacc

    AF = mybir.ActivationFunctionType
    OP = mybir.AluOpType
    F32 = mybir.dt.float32
    _dt = {"f32": F32, "f16": mybir.dt.float16, "bf16": mybir.dt.bfloat16}
    WDT = _dt[W_MODE]
    RDT = _dt[R_MODE]

    nc = bacc.Bacc("TRN2", target_bir_lowering=False, debug=False, num_devices=NCORES)

    xT = nc.dram_tensor("xT", (D, TOK), F32, kind="ExternalInput").ap()
    w1 = nc.dram_tensor("w1", (2, L1_NK, P, 512), WDT, kind="ExternalInput").ap()
    w2 = nc.dram_tensor("w2", (L2_NK, P, O), WDT, kind="ExternalInput").ap()
    out = nc.dram_tensor("out", (TOK, O), F32, kind="ExternalOutput").ap()

    with tile.TileContext(nc) as tc, ExitStack() as ctx:
        persist = ctx.enter_context(tc.tile_pool(name="persist", bufs=1))
        sp = ctx.enter_context(tc.tile_pool(name="sp", bufs=3))
        rp = ctx.enter_context(tc.tile_pool(name="rp", bufs=5))
        dp = ctx.enter_context(tc.tile_pool(name="dp", bufs=5))
        tp = ctx.enter_context(tc.tile_pool(name="tp", bufs=4))
        w1p = ctx.enter_context(tc.tile_pool(name="w1p", bufs=6))
        w2p = ctx.enter_context(tc.tile_pool(name="w2p", bufs=6))
        outp = ctx.enter_context(tc.tile_pool(name="outp", bufs=4))
        psum = ctx.enter_context(tc.tile_pool(name="psum", bufs=1, space="PSUM"))

        _bias_cache: dict = {}

        def bias_ap(val: float):
            if val not in _bias_cache:
                t = persist.tile([P, 1], F32, tag=f"bias{len(_bias_cache)}",
                                 name=f"bias_{len(_bias_cache)}")
                nc.vector.memset(t, val)
                _bias_cache[val] = t
            return _bias_cache[val]

        def chain(src, pref, dfac):
            """Yield D_0..D_8 [P, TOK] tiles computed from src (f32/PSUM).

            s_m/q_m evaluate relu/square of (2.5*src + 5.5 - m) on ACT;
            r/d/t/D run on DVE.  dfac(g) allocates the D output tile.
            """
            r: dict = {}
            d: dict = {}

            # clamp x at 2.2 (u at 11) so r_11 == 0 exactly and the d_10 =
            # r_10 alias below is valid; the true spline vanishes for u >= 11
            # (all B-splines end at 11), so this is exact, and u < 0 needs no
            # clamp (every relu(u-m) is already 0 there).
            xc = sp.tile([P, TOK], F32, tag="xc", name=f"xc{pref}")
            nc.vector.tensor_scalar_min(xc, src, 2.2)

            def mk_r(m):
                s = sp.tile([P, TOK], F32, tag="s", name=f"s{pref}m{m}")
                nc.scalar.activation(s, xc, AF.Relu, bias=bias_ap(5.5 - m), scale=2.5)
                q = sp.tile([P, TOK], F32, tag="q", name=f"q{pref}m{m}")
                nc.scalar.activation(q, xc, AF.Square, bias=bias_ap(5.5 - m), scale=2.5)
                t = rp.tile([P, TOK], F32, tag="r", name=f"r{pref}m{m}")
                nc.vector.tensor_mul(t, q, s)
                return t

            def mk_d(m):  # d_m = r_m - r_{m+1}; d_10 aliases r_10 (r_11 == 0)
                if m == 10:
                    return r[10]
                t = dp.tile([P, TOK], F32, tag="d", name=f"d{pref}m{m}")
                nc.vector.tensor_sub(t, r[m], r[m + 1])
                return t

            f: dict = {}
            for m in range(3):
                r[m] = mk_r(m)
            d[0] = mk_d(0)
            d[1] = mk_d(1)
            f[0] = tp.tile([P, TOK], F32, tag="f", name=f"f{pref}m0")
            nc.vector.tensor_sub(f[0], d[0], d[1])
            for g in range(NP):
                if g + 3 <= 10:
                    r[g + 3] = mk_r(g + 3)
                d[g + 2] = mk_d(g + 2)
                f[g + 1] = tp.tile([P, TOK], F32, tag="f", name=f"f{pref}m{g + 1}")
                nc.vector.tensor_sub(f[g + 1], d[g + 1], d[g + 2])
                Dg = dfac(g)
                nc.vector.tensor_sub(Dg, f[g], f[g + 1])
                yield Dg

        # ---- L1 inputs + silu planes ------------------------------------
        xt, si1 = [], []
        for ib in range(4):
            t = persist.tile([P, TOK], F32, tag=f"xt{ib}", name=f"xt{ib}")
            nc.sync.dma_start(out=t, in_=xT[ib * P : (ib + 1) * P, :])
            xt.append(t)
            s = persist.tile([P, TOK], RDT, tag=f"si1{ib}", name=f"si1_{ib}")
            nc.scalar.activation(s, t, AF.Silu, bias=bias_ap(0.0))
            si1.append(s)

        pb = [psum.tile([P, TOK], F32, tag=f"p{b}", name=f"p{b}") for b in range(8)]

        def l1_stream():
            for f in si1:
                yield f
            for ib in range(4):
                dfac = lambda g, ib=ib: persist.tile(
                    [P, TOK], RDT, tag=f"D1i{ib}g{g}", name=f"D1i{ib}g{g}"
                )
                yield from chain(xt[ib], f"a{ib}", dfac)

        # ---- L1 phase A: hidden cols 0..511 (banks 0..3), features JIT --
        feats = []
        gen = l1_stream()
        for k in range(L1_NK):
            f = next(gen)
            feats.append(f)
            wt = w1p.tile([P, 512], WDT, tag="w1k", name=f"w1a{k}")
            nc.sync.dma_start(out=wt, in_=w1[0, k])
            for ob in range(4):
                nc.tensor.matmul(
                    pb[ob], wt[:, ob * P : (ob + 1) * P], f,
                    start=(k == 0), stop=(k == L1_NK - 1),
                )
        # ---- L1 phase B: hidden cols 512..1023 (banks 4..7), reuse feats
        for k in range(L1_NK):
            wt = w1p.tile([P, 512], WDT, tag="w1k", name=f"w1b{k}")
            nc.sync.dma_start(out=wt, in_=w1[1, k])
            for ob in range(4):
                nc.tensor.matmul(
                    pb[4 + ob], wt[:, ob * P : (ob + 1) * P], feats[k],
                    start=(k == 0), stop=(k == L1_NK - 1),
                )

        dbg = os.environ.get("KAN_DEBUG", "")
        if dbg in ("l1h", "l1hb"):
            off = 0 if dbg == "l1h" else 4
            for b in range(4):
                ot = outp.tile([P, TOK], F32, tag="ot", name=f"dbg{b}")
                nc.vector.tensor_copy(ot, pb[off + b])
                nc.sync.dma_start(out=out[b * P : (b + 1) * P, :], in_=ot)

        # ---- L2 feature chains (emitted before any qb write: qb banks
        # alias pb[0..3], so every pb read must precede the first L2 MM) ---
        si2, d2 = [], {}
        for j in range(8):
            s = persist.tile([P, TOK], RDT, tag=f"si2{j}", name=f"si2_{j}")
            nc.scalar.activation(s, pb[j], AF.Silu, bias=bias_ap(0.0))
            si2.append(s)
            dfac = lambda g, j=j: persist.tile(
                [P, TOK], RDT, tag=f"D2j{j}g{g}", name=f"D2j{j}g{g}"
            )
            for g, Dg in enumerate(chain(pb[j], f"b{j}", dfac)):
                d2[(j, g)] = Dg

        # ---- L2 matmuls: out2[tok_blk, o] over 88 K-tiles, j-group-major
        qb = [psum.tile([P, O], F32, tag=f"p{tb}", name=f"q{tb}") for tb in range(4)]
        for j in range(8):
            k0 = j * (1 + NP)
            for c in range(1 + NP):
                k = k0 + c
                lhs = si2[j] if c == 0 else d2[(j, c - 1)]
                wt = w2p.tile([P, O], WDT, tag="w2k", name=f"w2k{k}")
                nc.sync.dma_start(out=wt, in_=w2[k])
                for tb in range(4):
                    nc.tensor.matmul(
                        qb[tb], lhs[:, tb * P : (tb + 1) * P], wt,
                        start=(k == 0), stop=(k == L2_NK - 1),
                    )

        # ---- store ------------------------------------------------------
        if not dbg:
            for tb in range(4):
                ot = outp.tile([P, O], F32, tag="ot", name=f"ot{tb}")
                nc.vector.tensor_copy(ot, qb[tb])
                nc.sync.dma_start(out=out[tb * P : (tb + 1) * P, :], in_=ot)

    nc.compile()
    _BUILD_CACHE["nc"] = nc
    return nc


# ---------------------------------------------------------------- entry
def kernel(x, coef1, scale_base1, scale_sp1, coef2, scale_base2, scale_sp2,
           _want_trace=False):
    from concourse.bass_utils import run_bass_kernel_spmd

    x_flat = np.asarray(x, np.float32).reshape(NTOK, D)
    w1 = _pack_w1(np.asarray(coef1), np.asarray(scale_sp1), np.asarray(scale_base1))
    w2 = _pack_w2(np.asarray(coef2), np.asarray(scale_sp2), np.asarray(scale_base2))

    nc = _build_kernel()

    in_maps = []
    for c in range(NCORES):
        xs = x_flat[c * TOK : (c + 1) * TOK]  # (TOK, D)
        in_maps.append(
            {
                "xT": np.ascontiguousarray(xs.T),
                "w1": w1,
                "w2": w2,
            }
        )

    res = run_bass_kernel_spmd(
        nc, in_maps, core_ids=list(range(NCORES)), trace=_want_trace
    )
    outs = [res.results[c]["out"] for c in range(NCORES)]
    full = np.concatenate(outs, axis=0).reshape(x.shape[0], x.shape[1], O)
    if _want_trace:
        kernel._last_results = res  # stash for test harness profiling
    return full.astype(np.float32)
